# revision 4
# baseline (speedup 1.0000x reference)
"""Fourier-basis temporal receptive field kernel for 8 TRN2 NeuronCores.

out[s,i,l,o] = sum_b phi_b(t[s,i,l]) * coefs[i,o,b], phi = interleaved
sin/cos Fourier basis + DC, data-parallel over nSeq (128 -> 16/core).

Host ships pre-range-reduced phases f' = frac(n*t/T + phase) - 0.5 in
fp16 (computed in float64); device does Sin ACT (basis = sin(-2pi f'),
arg in [-pi, pi] where the HW spline is valid).

Parity packing: per channel the basis tile holds K=128 rows = [64 basis
rows at even-l points; 64 basis rows at odd-l points] and the moving
matrix is parity-block-diagonal [[cp, 0], [0, cp]], so one matmul
yields PSUM [128 point-pairs, (l-parity, o)] whose partitions each
hold TWO consecutive l values x 64 outputs = 512 B of DRAM-contiguous
data.  512B DMA elements run at full bus rate (sub-512B elements are
penalized 2x), halving store queue time vs the naive [l, s, o] layout.
Stores (2 per channel, one per s-parity ph since DMA APs are limited
to 3 dims) are spread across all three DMA queues (sync/scalar HWDGE +
gpsimd SWDGE), scalar taking the fewest since it also runs ACT; the
DVE adds the DC plane (step-0 broadcast AP) while copying PSUM->SBUF.
"""

import numpy as np

import concourse.bass as bass
import concourse.tile as tile
from concourse import bacc, mybir
from concourse.bass_utils import run_bass_kernel_spmd

NCORES = 8
S, I, L, O = 128, 32, 128, 64
SL = S // NCORES          # 16 sequences per core
T = 127.0
F = SL * L                # 2048 points per channel per core
HF = F // 2               # 1024 point-pairs per channel
NPAIR = I // 2
NCH = I

# fr-load queue per pair, store queue per (channel, ph) half
_FRQ = ['gpsimd', 'sync'] * 8
_STQ = [('sync', 'gpsimd'), ('sync', 'scalar')] * 16

_CACHE: dict = {}


def _build():
    f32 = mybir.dt.float32
    f16 = mybir.dt.float16
    Sin = mybir.ActivationFunctionType.Sin
    nc = bacc.Bacc("TRN2", target_bir_lowering=False, debug=False,
                   num_devices=NCORES)
    fr_d = nc.dram_tensor("fr", [NPAIR, 128, F], f16,
                          kind="ExternalInput").ap()
    cpd_d = nc.dram_tensor("cpd", [128, NCH * 128], f16,
                           kind="ExternalInput").ap()
    dcb_d = nc.dram_tensor("dcb", [128, I * O], f32,
                           kind="ExternalInput").ap()
    out_d = nc.dram_tensor("out", [SL, I, L, O], f32,
                           kind="ExternalOutput").ap()

    with tile.TileContext(nc) as tc:
        with (
            tc.tile_pool(name="const", bufs=1) as constp,
            tc.tile_pool(name="frh", bufs=3) as frhp,
            tc.tile_pool(name="cb", bufs=3) as cbp,
            tc.tile_pool(name="stg", bufs=4) as stgp,
            tc.tile_pool(name="po", bufs=2, space=bass.MemorySpace.PSUM) as pop,
        ):
            cpd = constp.tile([128, NCH * 128], f16)
            dcb = constp.tile([128, I * O], f32)
            nc.sync.dma_start(cpd[:], cpd_d[:])
            nc.scalar.dma_start(dcb[:], dcb_d[:])

            for j in range(NPAIR):
                frh = frhp.tile([128, F], f16)
                getattr(nc, _FRQ[j]).dma_start(frh[:], fr_d[j])
                cb = cbp.tile([128, F], f16)
                for c in range(2):
                    nc.scalar.activation(
                        cb[:, c * HF:(c + 1) * HF],
                        frh[:, c * HF:(c + 1) * HF],
                        Sin, scale=-2.0 * np.pi)

                po = pop.tile([128, F], f32)
                for c in range(2):
                    ich = 2 * j + c
                    for ci in range(8):
                        lo = c * HF + ci * 128
                        nc.tensor.matmul(po[:, lo:lo + 128],
                                         cb[:, lo:lo + 128],
                                         cpd[:, ich * 128:(ich + 1) * 128],
                                         start=True, stop=True)

                stg = stgp.tile([128, F], f32)
                for c in range(2):
                    ich = 2 * j + c
                    sl_c = slice(c * HF, (c + 1) * HF)
                    ds = dcb[:, ich * O:(ich + 1) * O].unsqueeze(1) \
                        .broadcast_to([128, 16, O])
                    nc.vector.tensor_tensor(
                        stg[:, sl_c].rearrange("p (x o) -> p x o", o=O),
                        po[:, sl_c].rearrange("p (x o) -> p x o", o=O),
                        ds, mybir.AluOpType.add)

                for c in range(2):
                    ich = 2 * j + c
                    # dst: out[2*ci+ph, ich, 2*pl+cp, o] -> [ph, pl, ci, (cp o)]
                    dst4 = out_d[:, ich, :, :].rearrange(
                        "(ci ph) (pl cp) o -> ph pl ci (cp o)", ph=2, cp=2)
                    for ph in range(2):
                        # src: point-pairs pl of s-parity ph -> [pl, ci, 512B]
                        src = stg[ph * 64:(ph + 1) * 64,
                                  c * HF:(c + 1) * HF].rearrange(
                            "pl (ci co) -> pl ci co", co=128)
                        q = getattr(nc, _STQ[ich][ph])
                        q.dma_start(dst4[ph], src)

    nc.compile()
    return nc


def _prep_inputs(x: np.ndarray, coefs: np.ndarray):
    x = np.asarray(x, dtype=np.float32)
    coefs = np.asarray(coefs, dtype=np.float32)
    scale = np.float32(1.0 / np.sqrt(np.float32(T / 2.0)))
    const0 = np.float32(scale / np.sqrt(np.float32(2.0)))

    nvec = (np.arange(64) // 2 + 1).astype(np.float32)
    phase = np.where(np.arange(64) % 2 == 1, 0.25, 0.0).astype(np.float32)

    cb = np.transpose(coefs, (2, 0, 1)).reshape(65, I * O)
    cp = (cb[1:65] * scale).astype(np.float16)
    dc = (cb[0] * const0).astype(np.float32)
    dcb = np.broadcast_to(dc, (128, I * O)).copy()

    # parity-block-diagonal moving matrices, one per channel
    cpd = np.zeros((128, NCH * 128), np.float16)
    for ich in range(NCH):
        blk = cp[:, ich * O:(ich + 1) * O]            # [64, 64]
        cpd[0:64, ich * 128:ich * 128 + 64] = blk
        cpd[64:128, ich * 128 + 64:(ich + 1) * 128] = blk

    t = np.ascontiguousarray(x[:, :, 0, :])                  # [S, I, L]
    u64 = (nvec[:, None, None, None].astype(np.float64) / T) \
        * t[None].astype(np.float64) + phase[:, None, None, None]
    fr_all = (u64 - np.floor(u64) - 0.5).astype(np.float16)  # [64, S, I, L]

    in_maps = []
    for c in range(NCORES):
        sl_ = slice(c * SL, (c + 1) * SL)
        fr = np.empty((NPAIR, 128, F), np.float16)
        for j in range(NPAIR):
            for ch in range(2):
                fp = fr_all[:, sl_, 2 * j + ch, :]    # [64, 16 s, 128 l]
                lo = ch * HF
                fr[j, 0:64, lo:lo + HF] = fp[:, :, 0::2].reshape(64, HF)
                fr[j, 64:128, lo:lo + HF] = fp[:, :, 1::2].reshape(64, HF)
        in_maps.append({
            "fr": np.ascontiguousarray(fr),
            "cpd": np.ascontiguousarray(cpd),
            "dcb": np.ascontiguousarray(dcb),
        })
    return in_maps


def run(x, coefs, trace=False, **trace_kwargs):
    if "nc" not in _CACHE:
        _CACHE["nc"] = _build()
    nc = _CACHE["nc"]
    in_maps = _prep_inputs(x, coefs)
    res = run_bass_kernel_spmd(nc, in_maps, core_ids=list(range(NCORES)),
                               trace=trace, **trace_kwargs)
    out = np.concatenate([res.results[c]["out"] for c in range(NCORES)],
                         axis=0)
    return out, res


def kernel(x, coefs):
    out, _ = run(x, coefs)
    return out


# revision 5
# speedup vs baseline: 1.1328x; 1.1328x over previous
"""Fourier-basis temporal receptive field kernel for 8 TRN2 NeuronCores.

out[s,i,l,o] = sum_b phi_b(t[s,i,l]) * coefs[i,o,b], phi = interleaved
sin/cos Fourier basis + DC, data-parallel over nSeq (128 -> 16/core).

Host ships pre-range-reduced phases f' = frac(n*t/T + phase) - 0.5 in
fp16 (computed in float64); device does Sin ACT (basis = sin(-2pi f'),
arg in [-pi, pi] where the HW spline is valid).

Parity packing: per channel the basis tile holds K=128 rows = [64 basis
rows at even-l points; 64 basis rows at odd-l points] and the moving
matrix is parity-block-diagonal [[cp, 0], [0, cp]], so one matmul
yields PSUM [128 point-pairs, (l-parity, o)] whose partitions each
hold TWO consecutive l values x 64 outputs = 512 B of DRAM-contiguous
data.  512B DMA elements run at full bus rate (sub-512B elements are
penalized 2x), halving store queue time vs the naive [l, s, o] layout.

Queue plan: fr loads ride gpsimd SWDGE two-pairs-per-DMA (desc-gen is
~1us and the transfer is async, so the engine never blocks on HBM
latency); stores (2 per channel, one per s-parity ph, 3-dim APs) go
mostly to sync HWDGE with gpsimd taking a quarter and scalar almost
none, since scalar is pinned by the 32 Sin ACT passes.  Deep tile
pools (stg x6, cb x4) absorb DMA completion-latency jitter from the
8-core HBM contention.  DVE adds the DC plane (fp16, step-0 broadcast)
while copying PSUM->SBUF.
"""

import numpy as np

import concourse.bass as bass
import concourse.tile as tile
from concourse import bacc, mybir
from concourse.bass_utils import run_bass_kernel_spmd

NCORES = 8
S, I, L, O = 128, 32, 128, 64
SL = S // NCORES          # 16 sequences per core
T = 127.0
F = SL * L                # 2048 points per channel per core
HF = F // 2               # 1024 point-pairs per channel
NPAIR = I // 2
NCH = I

# store queue per (channel, ph) half: cycle of 8 channels
_STQ8 = [('sync', 'gpsimd'), ('sync', 'sync'), ('sync', 'gpsimd'),
         ('sync', 'sync'), ('sync', 'gpsimd'), ('sync', 'sync'),
         ('sync', 'gpsimd'), ('scalar', 'gpsimd')]
_STQ = _STQ8 * 4

_CACHE: dict = {}


def _build():
    f32 = mybir.dt.float32
    f16 = mybir.dt.float16
    Sin = mybir.ActivationFunctionType.Sin
    nc = bacc.Bacc("TRN2", target_bir_lowering=False, debug=False,
                   num_devices=NCORES)
    fr_d = nc.dram_tensor("fr", [NPAIR // 2, 128, 2 * F], f16,
                          kind="ExternalInput").ap()
    cpd_d = nc.dram_tensor("cpd", [128, NCH * 128], f16,
                           kind="ExternalInput").ap()
    dcb_d = nc.dram_tensor("dcb", [128, NCH * 128], f16,
                           kind="ExternalInput").ap()
    out_d = nc.dram_tensor("out", [SL, I, L, O], f32,
                           kind="ExternalOutput").ap()

    with tile.TileContext(nc) as tc:
        with (
            tc.tile_pool(name="const", bufs=1) as constp,
            tc.tile_pool(name="frh", bufs=3) as frhp,
            tc.tile_pool(name="cb", bufs=4) as cbp,
            tc.tile_pool(name="stg", bufs=6) as stgp,
            tc.tile_pool(name="po", bufs=2, space=bass.MemorySpace.PSUM) as pop,
        ):
            cpd = constp.tile([128, NCH * 128], f16)
            dcb = constp.tile([128, NCH * 128], f16)
            nc.sync.dma_start(cpd[:], cpd_d[:])
            nc.sync.dma_start(dcb[:], dcb_d[:])

            for j in range(NPAIR):
                if j % 2 == 0:
                    frh = frhp.tile([128, 2 * F], f16)
                    nc.gpsimd.dma_start(frh[:], fr_d[j // 2])
                fro = (j % 2) * F
                cb = cbp.tile([128, F], f16)
                for c in range(2):
                    nc.scalar.activation(
                        cb[:, c * HF:(c + 1) * HF],
                        frh[:, fro + c * HF:fro + (c + 1) * HF],
                        Sin, scale=-2.0 * np.pi)

                po = pop.tile([128, F], f32)
                for c in range(2):
                    ich = 2 * j + c
                    for ci in range(8):
                        lo = c * HF + ci * 128
                        nc.tensor.matmul(po[:, lo:lo + 128],
                                         cb[:, lo:lo + 128],
                                         cpd[:, ich * 128:(ich + 1) * 128],
                                         start=True, stop=True)

                stg = stgp.tile([128, F], f32)
                for c in range(2):
                    ich = 2 * j + c
                    sl_c = slice(c * HF, (c + 1) * HF)
                    ds = dcb[:, ich * 128:(ich + 1) * 128].unsqueeze(1) \
                        .broadcast_to([128, 8, 128])
                    nc.vector.tensor_tensor(
                        stg[:, sl_c].rearrange("p (x co) -> p x co", co=128),
                        po[:, sl_c].rearrange("p (x co) -> p x co", co=128),
                        ds, mybir.AluOpType.add)

                for c in range(2):
                    ich = 2 * j + c
                    # dst: out[2*ci+ph, ich, 2*pl+cp, o] -> [ph, pl, ci, (cp o)]
                    dst4 = out_d[:, ich, :, :].rearrange(
                        "(ci ph) (pl cp) o -> ph pl ci (cp o)", ph=2, cp=2)
                    for ph in range(2):
                        # src: point-pairs pl of s-parity ph -> [pl, ci, 512B]
                        src = stg[ph * 64:(ph + 1) * 64,
                                  c * HF:(c + 1) * HF].rearrange(
                            "pl (ci co) -> pl ci co", co=128)
                        q = getattr(nc, _STQ[ich][ph])
                        q.dma_start(dst4[ph], src)

    nc.compile()
    return nc


def _prep_inputs(x: np.ndarray, coefs: np.ndarray):
    x = np.asarray(x, dtype=np.float32)
    coefs = np.asarray(coefs, dtype=np.float32)
    scale = np.float32(1.0 / np.sqrt(np.float32(T / 2.0)))
    const0 = np.float32(scale / np.sqrt(np.float32(2.0)))

    nvec = (np.arange(64) // 2 + 1).astype(np.float32)
    phase = np.where(np.arange(64) % 2 == 1, 0.25, 0.0).astype(np.float32)

    cb = np.transpose(coefs, (2, 0, 1)).reshape(65, I * O)
    cp = (cb[1:65] * scale).astype(np.float16)
    dc = (cb[0] * const0).astype(np.float16)          # [I*O]

    # parity-block-diagonal moving matrices + DC tiled over l-parity
    cpd = np.zeros((128, NCH * 128), np.float16)
    dcb = np.empty((128, NCH * 128), np.float16)
    for ich in range(NCH):
        blk = cp[:, ich * O:(ich + 1) * O]            # [64, 64]
        cpd[0:64, ich * 128:ich * 128 + 64] = blk
        cpd[64:128, ich * 128 + 64:(ich + 1) * 128] = blk
        dcb[:, ich * 128:ich * 128 + 64] = dc[ich * O:(ich + 1) * O]
        dcb[:, ich * 128 + 64:(ich + 1) * 128] = dc[ich * O:(ich + 1) * O]

    t = np.ascontiguousarray(x[:, :, 0, :])                  # [S, I, L]
    u64 = (nvec[:, None, None, None].astype(np.float64) / T) \
        * t[None].astype(np.float64) + phase[:, None, None, None]
    fr_all = (u64 - np.floor(u64) - 0.5).astype(np.float16)  # [64, S, I, L]

    in_maps = []
    for c in range(NCORES):
        sl_ = slice(c * SL, (c + 1) * SL)
        fr = np.empty((NPAIR // 2, 128, 2 * F), np.float16)
        for j in range(NPAIR):
            for ch in range(2):
                fp = fr_all[:, sl_, 2 * j + ch, :]    # [64, 16 s, 128 l]
                lo = (j % 2) * F + ch * HF
                fr[j // 2, 0:64, lo:lo + HF] = fp[:, :, 0::2].reshape(64, HF)
                fr[j // 2, 64:128, lo:lo + HF] = fp[:, :, 1::2].reshape(64, HF)
        in_maps.append({
            "fr": np.ascontiguousarray(fr),
            "cpd": np.ascontiguousarray(cpd),
            "dcb": np.ascontiguousarray(dcb),
        })
    return in_maps


def run(x, coefs, trace=False, **trace_kwargs):
    if "nc" not in _CACHE:
        _CACHE["nc"] = _build()
    nc = _CACHE["nc"]
    in_maps = _prep_inputs(x, coefs)
    res = run_bass_kernel_spmd(nc, in_maps, core_ids=list(range(NCORES)),
                               trace=trace, **trace_kwargs)
    out = np.concatenate([res.results[c]["out"] for c in range(NCORES)],
                         axis=0)
    return out, res


def kernel(x, coefs):
    out, _ = run(x, coefs)
    return out


# revision 6
# speedup vs baseline: 1.2417x; 1.0961x over previous
"""Fourier-basis temporal receptive field kernel for 8 TRN2 NeuronCores.

out[s,i,l,o] = sum_b phi_b(t[s,i,l]) * coefs[i,o,b], phi = interleaved
sin/cos Fourier basis + DC, data-parallel over nSeq (128 -> 16/core).

The kernel is DMA-bound (16.8 MB of fp32 output per core), so the
basis is generated on-device instead of shipping 8.4 MB of host
phases: per channel a single K=28 matmul computes the range-reduced
phase -frac'(n t/T + phase) via the magic-number trick, relying on the
PE's sequential fp32 accumulation down the K rows:

  rows  0-11  + w-part x t-part products (exact bf16 splits)
  row     12  + phase (0 / 0.25 for sin/cos rows)
  row     13  + MAGIC (1.5*2^23)  -> fp32 accumulator rounds to int
  row     14  - MAGIC             -> round(u)
  rows 15-27  - phase, - products -> round(u) - u = -frac' in [-.5,.5]

Sin ACT then gives basis = sin(-2pi x) (HW spline valid in [-pi,pi]).

Parity packing: per channel the basis tile holds K=128 rows = [64 basis
rows at even-l points; 64 at odd-l points] and the coef moving matrix
is parity-block-diagonal [[cp, 0], [0, cp]], so each main matmul
yields PSUM [128 point-pairs, (l-parity, o)] whose partitions hold TWO
consecutive l values x 64 outputs = 512 B of DRAM-contiguous data
(sub-512B DMA elements run at half bus rate).  DVE adds the DC plane
(fp16, step-0 broadcast) while copying PSUM->SBUF; stores (2 per
channel, one per s-parity, 3-dim APs) spread over sync HWDGE mostly,
gpsimd SWDGE (async transfers), scalar least (it runs the ACTs).
"""

import numpy as np
import ml_dtypes

import concourse.bass as bass
import concourse.tile as tile
from concourse import bacc, mybir
from concourse.bass_utils import run_bass_kernel_spmd

NCORES = 8
S, I, L, O = 128, 32, 128, 64
SL = S // NCORES          # 16 sequences per core
T = 127.0
F = SL * L                # 2048 points per channel per core
HF = F // 2               # 1024 point-pairs per channel
NPAIR = I // 2
NCH = I
KA = 28                   # angle-MM rows
MAGIC = np.float32(1.5 * 2 ** 23)
SINGLE_MM = True          # one K=28 angle matmul vs two K=14 halves

# store queue per (channel, ph) half: cycle of 8 channels
_STQ8 = [('sync', 'gpsimd'), ('sync', 'sync'), ('sync', 'gpsimd'),
         ('sync', 'scalar'), ('sync', 'gpsimd'), ('sync', 'sync'),
         ('sync', 'gpsimd'), ('scalar', 'gpsimd')]
_STQ = _STQ8 * 4

_CACHE: dict = {}


def _build():
    f32 = mybir.dt.float32
    f16 = mybir.dt.float16
    bf16 = mybir.dt.bfloat16
    Sin = mybir.ActivationFunctionType.Sin
    nc = bacc.Bacc("TRN2", target_bir_lowering=False, debug=False,
                   num_devices=NCORES)
    tw_d = nc.dram_tensor("tw", [NPAIR, KA, F], bf16,
                          kind="ExternalInput").ap()
    spA_d = nc.dram_tensor("spA", [KA, 128], bf16, kind="ExternalInput").ap()
    cpd_d = nc.dram_tensor("cpd", [128, NCH * 128], f16,
                           kind="ExternalInput").ap()
    dcb_d = nc.dram_tensor("dcb", [128, NCH * 128], f16,
                           kind="ExternalInput").ap()
    out_d = nc.dram_tensor("out", [SL, I, L, O], f32,
                           kind="ExternalOutput").ap()

    with tile.TileContext(nc) as tc:
        with (
            tc.tile_pool(name="const", bufs=1) as constp,
            tc.tile_pool(name="tw", bufs=3) as twp,
            tc.tile_pool(name="cb", bufs=4) as cbp,
            tc.tile_pool(name="stg", bufs=8) as stgp,
            tc.tile_pool(name="ang", bufs=2, space=bass.MemorySpace.PSUM) as angp,
            tc.tile_pool(name="po", bufs=2, space=bass.MemorySpace.PSUM) as pop,
        ):
            spA = constp.tile([KA, 128], bf16)
            cpd = constp.tile([128, NCH * 128], f16)
            dcb = constp.tile([128, NCH * 128], f16)
            nc.sync.dma_start(spA[:], spA_d[:])
            nc.sync.dma_start(cpd[:], cpd_d[:])
            nc.sync.dma_start(dcb[:], dcb_d[:])

            for ich in range(NCH):
                j, c = divmod(ich, 2)
                if c == 0:
                    twt = twp.tile([KA, F], bf16)
                    nc.gpsimd.dma_start(twt[:], tw_d[j])

                ang = angp.tile([128, HF], f32)
                for h in range(2):
                    sl_h = slice(c * HF + h * 512, c * HF + (h + 1) * 512)
                    acc = ang[:, h * 512:(h + 1) * 512]
                    if SINGLE_MM:
                        nc.tensor.matmul(acc, spA[:], twt[:, sl_h],
                                         start=True, stop=True)
                    else:
                        nc.tensor.matmul(acc, spA[0:14, :],
                                         twt[0:14, sl_h],
                                         start=True, stop=False)
                        nc.tensor.matmul(acc, spA[14:28, :],
                                         twt[14:28, sl_h],
                                         start=False, stop=True)

                cb = cbp.tile([128, HF], f16)
                nc.scalar.activation(cb[:], ang[:], Sin, scale=-2.0 * np.pi)

                po = pop.tile([128, HF], f32)
                for ci in range(8):
                    nc.tensor.matmul(po[:, ci * 128:(ci + 1) * 128],
                                     cb[:, ci * 128:(ci + 1) * 128],
                                     cpd[:, ich * 128:(ich + 1) * 128],
                                     start=True, stop=True)

                stg = stgp.tile([128, HF], f32)
                ds = dcb[:, ich * 128:(ich + 1) * 128].unsqueeze(1) \
                    .broadcast_to([128, 8, 128])
                nc.vector.tensor_tensor(
                    stg[:].rearrange("p (x co) -> p x co", co=128),
                    po[:].rearrange("p (x co) -> p x co", co=128),
                    ds, mybir.AluOpType.add)

                # dst: out[2*ci+ph, ich, 2*pl+cp, o] -> [ph, pl, ci, (cp o)]
                dst4 = out_d[:, ich, :, :].rearrange(
                    "(ci ph) (pl cp) o -> ph pl ci (cp o)", ph=2, cp=2)
                for ph in range(2):
                    # src: point-pairs pl of s-parity ph -> [pl, ci, 512B]
                    src = stg[ph * 64:(ph + 1) * 64, :].rearrange(
                        "pl (ci co) -> pl ci co", co=128)
                    q = getattr(nc, _STQ[ich][ph])
                    q.dma_start(dst4[ph], src)

    nc.compile()
    return nc


def _split3(a):
    """Split fp32 array into three bf16 parts summing (nearly) exactly."""
    h = a.astype(ml_dtypes.bfloat16).astype(np.float32)
    r = a - h
    m = r.astype(ml_dtypes.bfloat16).astype(np.float32)
    l = (r - m).astype(ml_dtypes.bfloat16).astype(np.float32)
    return h, m, l


def _prep_inputs(x: np.ndarray, coefs: np.ndarray):
    x = np.asarray(x, dtype=np.float32)
    coefs = np.asarray(coefs, dtype=np.float32)
    scale = np.float32(1.0 / np.sqrt(np.float32(T / 2.0)))
    const0 = np.float32(scale / np.sqrt(np.float32(2.0)))

    nvec = (np.arange(64) // 2 + 1).astype(np.float32)
    w = nvec / np.float32(T)
    wh, wm, wl = _split3(w)
    phase = np.where(np.arange(64) % 2 == 1, 0.25, 0.0).astype(np.float32)
    wrows = np.stack([wh, wh, wh, wm, wm, wl])               # [6, 64]
    ph2 = np.concatenate([phase, phase])                     # [128]

    # stationary spA [28, 128], cols = (parity, basis k)
    spA = np.zeros((KA, 128), np.float32)
    spA[0:6, 0:64] = wrows
    spA[6:12, 64:128] = wrows
    spA[12, :] = ph2
    spA[13, :] = MAGIC
    spA[14, :] = -MAGIC
    spA[15, :] = -ph2
    spA[16:22, 0:64] = -wrows
    spA[22:28, 64:128] = -wrows
    to_bf = lambda a: np.ascontiguousarray(a).astype(ml_dtypes.bfloat16)

    cbt = np.transpose(coefs, (2, 0, 1)).reshape(65, I * O)
    cp = (cbt[1:65] * scale).astype(np.float16)
    dc = (cbt[0] * const0).astype(np.float16)                # [I*O]

    cpd = np.zeros((128, NCH * 128), np.float16)
    dcb = np.empty((128, NCH * 128), np.float16)
    for ich in range(NCH):
        blk = cp[:, ich * O:(ich + 1) * O]                   # [64, 64]
        cpd[0:64, ich * 128:ich * 128 + 64] = blk
        cpd[64:128, ich * 128 + 64:(ich + 1) * 128] = blk
        dcb[:, ich * 128:ich * 128 + 64] = dc[ich * O:(ich + 1) * O]
        dcb[:, ich * 128 + 64:(ich + 1) * 128] = dc[ich * O:(ich + 1) * O]

    t = np.ascontiguousarray(x[:, :, 0, :])                  # [S, I, L]

    in_maps = []
    for core in range(NCORES):
        sl_ = slice(core * SL, (core + 1) * SL)
        tw = np.ones((NPAIR, KA, F), np.float32)
        for j in range(NPAIR):
            for c in range(2):
                tc_ = t[sl_, 2 * j + c, :]                   # [16 s, 128 l]
                te = np.ascontiguousarray(tc_[:, 0::2]).reshape(HF)
                to = np.ascontiguousarray(tc_[:, 1::2]).reshape(HF)
                eh, em, el = _split3(te)
                oh, om, ol = _split3(to)
                lo = c * HF
                for k, arr in enumerate((eh, em, el, eh, em, eh)):
                    tw[j, k, lo:lo + HF] = arr
                    tw[j, 16 + k, lo:lo + HF] = arr
                for k, arr in enumerate((oh, om, ol, oh, om, oh)):
                    tw[j, 6 + k, lo:lo + HF] = arr
                    tw[j, 22 + k, lo:lo + HF] = arr
        in_maps.append({
            "tw": to_bf(tw),
            "spA": to_bf(spA),
            "cpd": np.ascontiguousarray(cpd),
            "dcb": np.ascontiguousarray(dcb),
        })
    return in_maps


def run(x, coefs, trace=False, **trace_kwargs):
    if "nc" not in _CACHE:
        _CACHE["nc"] = _build()
    nc = _CACHE["nc"]
    in_maps = _prep_inputs(x, coefs)
    res = run_bass_kernel_spmd(nc, in_maps, core_ids=list(range(NCORES)),
                               trace=trace, **trace_kwargs)
    out = np.concatenate([res.results[c]["out"] for c in range(NCORES)],
                         axis=0)
    return out, res


def kernel(x, coefs):
    out, _ = run(x, coefs)
    return out


# revision 8
# speedup vs baseline: 1.2961x; 1.0438x over previous
"""Fourier-basis temporal receptive field kernel for 8 TRN2 NeuronCores.

out[s,i,l,o] = sum_b phi_b(t[s,i,l]) * coefs[i,o,b], phi = interleaved
sin/cos Fourier basis + DC, data-parallel over nSeq (128 -> 16/core).

The kernel is DMA-bound (16.8 MB of fp32 output per core), so the
basis is generated on-device instead of shipping 8.4 MB of host
phases: per channel a single K=28 matmul computes the range-reduced
phase -frac'(n t/T + phase) via the magic-number trick, relying on the
PE's sequential fp32 accumulation down the K rows:

  rows  0-11  + w-part x t-part products (exact bf16 splits)
  row     12  + phase (0 / 0.25 for sin/cos rows)
  row     13  + MAGIC (1.5*2^23)  -> fp32 accumulator rounds to int
  row     14  - MAGIC             -> round(u)
  rows 15-27  - phase, - products -> round(u) - u = -frac' in [-.5,.5]

Sin ACT then gives basis = sin(-2pi x) (HW spline valid in [-pi,pi]).

Parity packing: per channel the basis tile holds K=128 rows = [64 basis
rows at even-l points; 64 at odd-l points] and the coef moving matrix
is parity-block-diagonal [[cp, 0], [0, cp]], so each main matmul
yields PSUM [128 point-pairs, (l-parity, o)] whose partitions hold TWO
consecutive l values x 64 outputs = 512 B of DRAM-contiguous data
(sub-512B DMA elements run at half bus rate).  DVE adds the DC plane
(fp16, step-0 broadcast) while copying PSUM->SBUF; stores (2 per
channel, one per s-parity, 3-dim APs) spread over sync HWDGE mostly,
gpsimd SWDGE (async transfers), scalar least (it runs the ACTs).
"""

import numpy as np
import ml_dtypes

import concourse.bass as bass
import concourse.tile as tile
from concourse import bacc, mybir
from concourse.bass_utils import run_bass_kernel_spmd

NCORES = 8
S, I, L, O = 128, 32, 128, 64
SL = S // NCORES          # 16 sequences per core
T = 127.0
F = SL * L                # 2048 points per channel per core
HF = F // 2               # 1024 point-pairs per channel
NPAIR = I // 2
NCH = I
KA = 28                   # angle-MM rows
MAGIC = np.float32(1.5 * 2 ** 23)
SINGLE_MM = True          # one K=28 angle matmul vs two K=14 halves

# store queue per (channel, ph) half; scalar-heavy at the end since it
# is done with ACT passes by then while sync/gpsimd drain their backlog
_STQ6 = [('sync', 'gpsimd'), ('sync', 'scalar'), ('gpsimd', 'sync'),
         ('sync', 'gpsimd'), ('scalar', 'sync'), ('sync', 'gpsimd')]
_STQ = _STQ6 * 4 + [('scalar', 'gpsimd'), ('scalar', 'sync')] * 4

_CACHE: dict = {}


def _build():
    f32 = mybir.dt.float32
    f16 = mybir.dt.float16
    bf16 = mybir.dt.bfloat16
    Sin = mybir.ActivationFunctionType.Sin
    nc = bacc.Bacc("TRN2", target_bir_lowering=False, debug=False,
                   num_devices=NCORES)
    tw_d = nc.dram_tensor("tw", [NPAIR, KA, F], bf16,
                          kind="ExternalInput").ap()
    spA_d = nc.dram_tensor("spA", [KA, 128], bf16, kind="ExternalInput").ap()
    cpd_d = nc.dram_tensor("cpd", [128, NCH * 128], f16,
                           kind="ExternalInput").ap()
    dcb_d = nc.dram_tensor("dcb", [128, NCH * 128], f16,
                           kind="ExternalInput").ap()
    out_d = nc.dram_tensor("out", [SL, I, L, O], f32,
                           kind="ExternalOutput").ap()

    with tile.TileContext(nc) as tc:
        with (
            tc.tile_pool(name="const", bufs=1) as constp,
            tc.tile_pool(name="tw", bufs=4) as twp,
            tc.tile_pool(name="cb", bufs=6) as cbp,
            tc.tile_pool(name="stg", bufs=10) as stgp,
            tc.tile_pool(name="ang", bufs=2, space=bass.MemorySpace.PSUM) as angp,
            tc.tile_pool(name="po", bufs=2, space=bass.MemorySpace.PSUM) as pop,
        ):
            spA = constp.tile([KA, 128], bf16)
            cpd = constp.tile([128, NCH * 128], f16)
            dcb = constp.tile([128, NCH * 128], f16)
            nc.sync.dma_start(spA[:], spA_d[:])
            nc.sync.dma_start(cpd[:], cpd_d[:])
            nc.sync.dma_start(dcb[:], dcb_d[:])

            for ich in range(NCH):
                j, c = divmod(ich, 2)
                if c == 0:
                    twt = twp.tile([KA, F], bf16)
                    nc.gpsimd.dma_start(twt[:], tw_d[j])

                ang = angp.tile([128, HF], f32)
                for h in range(2):
                    sl_h = slice(c * HF + h * 512, c * HF + (h + 1) * 512)
                    acc = ang[:, h * 512:(h + 1) * 512]
                    if SINGLE_MM:
                        nc.tensor.matmul(acc, spA[:], twt[:, sl_h],
                                         start=True, stop=True)
                    else:
                        nc.tensor.matmul(acc, spA[0:14, :],
                                         twt[0:14, sl_h],
                                         start=True, stop=False)
                        nc.tensor.matmul(acc, spA[14:28, :],
                                         twt[14:28, sl_h],
                                         start=False, stop=True)

                cb = cbp.tile([128, HF], f16)
                nc.scalar.activation(cb[:], ang[:], Sin, scale=-2.0 * np.pi)

                po = pop.tile([128, HF], f32)
                for ci in range(8):
                    nc.tensor.matmul(po[:, ci * 128:(ci + 1) * 128],
                                     cb[:, ci * 128:(ci + 1) * 128],
                                     cpd[:, ich * 128:(ich + 1) * 128],
                                     start=True, stop=True)

                stg = stgp.tile([128, HF], f32)
                ds = dcb[:, ich * 128:(ich + 1) * 128].unsqueeze(1) \
                    .broadcast_to([128, 8, 128])
                nc.vector.tensor_tensor(
                    stg[:].rearrange("p (x co) -> p x co", co=128),
                    po[:].rearrange("p (x co) -> p x co", co=128),
                    ds, mybir.AluOpType.add)

                # dst: out[2*ci+ph, ich, 2*pl+cp, o] -> [ph, pl, ci, (cp o)]
                dst4 = out_d[:, ich, :, :].rearrange(
                    "(ci ph) (pl cp) o -> ph pl ci (cp o)", ph=2, cp=2)
                for ph in range(2):
                    # src: point-pairs pl of s-parity ph -> [pl, ci, 512B]
                    src = stg[ph * 64:(ph + 1) * 64, :].rearrange(
                        "pl (ci co) -> pl ci co", co=128)
                    q = getattr(nc, _STQ[ich][ph])
                    q.dma_start(dst4[ph], src)

    nc.compile()
    return nc


def _split3(a):
    """Split fp32 array into three bf16 parts summing (nearly) exactly."""
    h = a.astype(ml_dtypes.bfloat16).astype(np.float32)
    r = a - h
    m = r.astype(ml_dtypes.bfloat16).astype(np.float32)
    l = (r - m).astype(ml_dtypes.bfloat16).astype(np.float32)
    return h, m, l


def _prep_inputs(x: np.ndarray, coefs: np.ndarray):
    x = np.asarray(x, dtype=np.float32)
    coefs = np.asarray(coefs, dtype=np.float32)
    scale = np.float32(1.0 / np.sqrt(np.float32(T / 2.0)))
    const0 = np.float32(scale / np.sqrt(np.float32(2.0)))

    nvec = (np.arange(64) // 2 + 1).astype(np.float32)
    w = nvec / np.float32(T)
    wh, wm, wl = _split3(w)
    phase = np.where(np.arange(64) % 2 == 1, 0.25, 0.0).astype(np.float32)
    wrows = np.stack([wh, wh, wh, wm, wm, wl])               # [6, 64]
    ph2 = np.concatenate([phase, phase])                     # [128]

    # stationary spA [28, 128], cols = (parity, basis k)
    spA = np.zeros((KA, 128), np.float32)
    spA[0:6, 0:64] = wrows
    spA[6:12, 64:128] = wrows
    spA[12, :] = ph2
    spA[13, :] = MAGIC
    spA[14, :] = -MAGIC
    spA[15, :] = -ph2
    spA[16:22, 0:64] = -wrows
    spA[22:28, 64:128] = -wrows
    to_bf = lambda a: np.ascontiguousarray(a).astype(ml_dtypes.bfloat16)

    cbt = np.transpose(coefs, (2, 0, 1)).reshape(65, I * O)
    cp = (cbt[1:65] * scale).astype(np.float16)
    dc = (cbt[0] * const0).astype(np.float16)                # [I*O]

    cpd = np.zeros((128, NCH * 128), np.float16)
    dcb = np.empty((128, NCH * 128), np.float16)
    for ich in range(NCH):
        blk = cp[:, ich * O:(ich + 1) * O]                   # [64, 64]
        cpd[0:64, ich * 128:ich * 128 + 64] = blk
        cpd[64:128, ich * 128 + 64:(ich + 1) * 128] = blk
        dcb[:, ich * 128:ich * 128 + 64] = dc[ich * O:(ich + 1) * O]
        dcb[:, ich * 128 + 64:(ich + 1) * 128] = dc[ich * O:(ich + 1) * O]

    t = np.ascontiguousarray(x[:, :, 0, :])                  # [S, I, L]

    in_maps = []
    for core in range(NCORES):
        sl_ = slice(core * SL, (core + 1) * SL)
        tw = np.ones((NPAIR, KA, F), np.float32)
        for j in range(NPAIR):
            for c in range(2):
                tc_ = t[sl_, 2 * j + c, :]                   # [16 s, 128 l]
                te = np.ascontiguousarray(tc_[:, 0::2]).reshape(HF)
                to = np.ascontiguousarray(tc_[:, 1::2]).reshape(HF)
                eh, em, el = _split3(te)
                oh, om, ol = _split3(to)
                lo = c * HF
                for k, arr in enumerate((eh, em, el, eh, em, eh)):
                    tw[j, k, lo:lo + HF] = arr
                    tw[j, 16 + k, lo:lo + HF] = arr
                for k, arr in enumerate((oh, om, ol, oh, om, oh)):
                    tw[j, 6 + k, lo:lo + HF] = arr
                    tw[j, 22 + k, lo:lo + HF] = arr
        in_maps.append({
            "tw": to_bf(tw),
            "spA": to_bf(spA),
            "cpd": np.ascontiguousarray(cpd),
            "dcb": np.ascontiguousarray(dcb),
        })
    return in_maps


def run(x, coefs, trace=False, **trace_kwargs):
    if "nc" not in _CACHE:
        _CACHE["nc"] = _build()
    nc = _CACHE["nc"]
    in_maps = _prep_inputs(x, coefs)
    res = run_bass_kernel_spmd(nc, in_maps, core_ids=list(range(NCORES)),
                               trace=trace, **trace_kwargs)
    out = np.concatenate([res.results[c]["out"] for c in range(NCORES)],
                         axis=0)
    return out, res


def kernel(x, coefs):
    out, _ = run(x, coefs)
    return out
